# revision 28
# baseline (speedup 1.0000x reference)
"""Trainium2 Bass kernel for EnhancedReconstructionLoss (0.8*MSE + 0.2*SSIM-loss).

Sharding: pure data parallel. Batch 32 -> 8 cores x 4 images (12 planes of
512x512 each). Each core computes partial sums (sum x^2, sum y^2, sum x*y,
sum ssim_map); host combines into the scalar loss.

v2 design (vs baseline):
  - Fully-packed input tiles [128, 4, 512]: tile0 = rows 0..126 + row 511 at
    partition 127; tiles 1..3 = rows 127..510. Every pointwise op runs at a
    clean FD=2048 with zero pad waste and exact plane-sum accumulators.
  - zz = xx+yy stream is never materialized: its box filter is computed by
    accumulating the xx and yy matmuls into the same PSUM bank.
  - Vertical 3-tap via banded matmuls (main 127/128-row band + tiny 2-row
    edge accumulation from the next tile; chunk-3 edge reads row 511 from
    tile0 partition 127 via a base-64 quadrant matmul).
  - PSUM->SBUF copies fold all SSIM constants: Sx,Sy scaled by sqrt(2) (so
    P2 = Sx'*Sy' = 2*Sx*Sy), xy scaled by 18 + bias 27*C2, zz scaled by 9 +
    bias 27*C2. After the horizontal taps: H = 18*Sxy+81*C2, G = 9*Szz+81*C2.
  - Tail per plane (FD 2048, all bf16 incl. the reciprocal, emitted via
    _custom_dve to skip the fp32-only wrapper check): qsum=qx+qy,
    den1=qsum+c1, den2=G-qsum, P2, num1=P2+c1, num2=H-P2, num=num1*num2,
    dd=den1*den2, r=recip_approx_fast(dd), sm=num*r, accum.  81^2 scaling
    cancels in the ratio, so ssim values are direct (no host-side x9).
  - Everything elementwise runs on Vector/Scalar only: GpSimd shares the
    SBUF port with the DVE and measurably slows concurrent Vector ops.
  - Emission order is software-pipelined: next plane's DMA + Scalar
    squares are emitted before this plane's tail; the xy pre-op (STT with
    MSE accumulator) is emitted mid-tail so Vector's in-order queue never
    head-of-line blocks on DMA; xy-dependent matmuls go last per chunk.
"""

import sys
import numpy as np

for _p in ("/opt/trn_rl_repo", "/root/.axon_site/_ro/trn_rl_repo"):
    if _p not in sys.path:
        sys.path.insert(0, _p)

N_CORES = 8
IMG = 512
PLANES = 12          # 4 images x 3 channels per core
NCHUNK = 4
C1 = 0.01 ** 2
C2 = 0.03 ** 2
c1 = 81.0 * C1       # folded constants (81 = 9^2 pool divisors, cancels)
c2 = 81.0 * C2
RT2 = float(np.sqrt(2.0))

CFG = {
    "dma_eng": "sync",
}

_compiled = None


def _build_nc():
    from contextlib import ExitStack
    import concourse.bass as bass
    import concourse.tile as tile
    from concourse import bacc, mybir

    f32 = mybir.dt.float32
    bf16 = mybir.dt.bfloat16
    Alu = mybir.AluOpType
    Act = mybir.ActivationFunctionType

    nc = bacc.Bacc("TRN2", target_bir_lowering=False, debug=False,
                   enable_asserts=True, num_devices=N_CORES)
    x_d = nc.dram_tensor("x", [PLANES, IMG, IMG], bf16, kind="ExternalInput").ap()
    y_d = nc.dram_tensor("y", [PLANES, IMG, IMG], bf16, kind="ExternalInput").ap()
    band_d = nc.dram_tensor("band", [321, 128], bf16, kind="ExternalInput").ap()
    out_d = nc.dram_tensor("out", [128, 4], f32, kind="ExternalOutput").ap()

    dma = getattr(nc, CFG["dma_eng"])

    with tile.TileContext(nc) as tc, ExitStack() as ctx:
        consts = ctx.enter_context(tc.tile_pool(name="consts", bufs=1))
        inp = ctx.enter_context(tc.tile_pool(name="inp", bufs=4))
        pre = ctx.enter_context(tc.tile_pool(name="pre", bufs=3))
        psum = ctx.enter_context(tc.tile_pool(name="psum", bufs=2, space="PSUM"))
        vsp = ctx.enter_context(tc.tile_pool(name="vsp", bufs=3))
        smap = ctx.enter_context(tc.tile_pool(name="smap", bufs=2))
        tshort = ctx.enter_context(tc.tile_pool(name="tshort", bufs=2))
        tmed = ctx.enter_context(tc.tile_pool(name="tmed", bufs=1))
        tapA = ctx.enter_context(tc.tile_pool(name="tapA", bufs=1))
        accs = ctx.enter_context(tc.tile_pool(name="accs", bufs=1))

        # band_a [128,128]: k-j in {0,1,2} (chunks 1..3 main)
        band_a = consts.tile([128, 128], bf16, tag="band_a")
        dma.dma_start(out=band_a, in_=band_d[0:128, :])
        # band_b [127,128]: k-j in {-1,0,1} (chunk 0 main)
        band_b = consts.tile([127, 128], bf16, tag="band_b")
        dma.dma_start(out=band_b, in_=band_d[128:255, :])
        # e2 [2,128]: next-tile rows -> outputs 126,127 (chunks 0..2 edge)
        e2 = consts.tile([2, 128], bf16, tag="e2")
        dma.dma_start(out=e2, in_=band_d[255:257, :])
        # e1b: row 511 (tile0 p127) -> outputs 510,511; lives at partitions
        # 64..127 so lhsT/rhs base partitions match (PE quadrant rule)
        e1b_full = consts.tile([128, 128], bf16, tag="e1b")
        dma.dma_start(out=e1b_full[64:128, :], in_=band_d[257:321, :])
        e1b = e1b_full[64:128, :]

        xxacc = accs.tile([128, PLANES], f32, tag="xxacc")
        yyacc = accs.tile([128, PLANES], f32, tag="yyacc")
        xyacc = accs.tile([128, PLANES], f32, tag="xyacc")
        ssacc = accs.tile([128, PLANES], f32, tag="ssacc")

        def load_plane(dst, src_d, p, eng):
            # tile 0: rows 0..126 at p0..126, row 511 at p127
            eng.dma_start(out=dst[0:127, 0, :], in_=src_d[p, 0:127, :])
            eng.dma_start(out=dst[127:128, 0, :], in_=src_d[p, 511:512, :])
            # tiles 1..2: rows 127..382
            mid = src_d[p, 127:383, :].rearrange("(t r) c -> r t c", r=128)
            eng.dma_start(out=dst[:, 1:3, :], in_=mid)
            # tile 3: rows 383..510
            eng.dma_start(out=dst[:, 3, :], in_=src_d[p, 383:511, :])

        def load_pre_s(p):
            xp = inp.tile([128, NCHUNK, IMG], bf16, tag="xp")
            yp = inp.tile([128, NCHUNK, IMG], bf16, tag="yp")
            load_plane(xp, x_d, p, dma)
            load_plane(yp, y_d, p, dma)
            # pre-pool pointwise; accumulators are exact plane sums (packed)
            xxp = pre.tile([128, NCHUNK, IMG], bf16, tag="xx")
            yyp = pre.tile([128, NCHUNK, IMG], bf16, tag="yy")
            nc.scalar.activation(xxp, xp, Act.Square,
                                 accum_out=xxacc[:, p:p + 1])
            nc.scalar.activation(yyp, yp, Act.Square,
                                 accum_out=yyacc[:, p:p + 1])
            return xp, yp, xxp, yyp

        def pre_v(st, p):
            xp, yp, xxp, yyp = st
            xyp = pre.tile([128, NCHUNK, IMG], bf16, tag="xy")
            nc.vector.scalar_tensor_tensor(
                out=xyp, in0=xp, scalar=1.0, in1=yp,
                op0=Alu.mult, op1=Alu.mult, accum_out=xyacc[:, p:p + 1])
            return xp, yp, xxp, yyp, xyp

        nxt = pre_v(load_pre_s(0), 0)
        for p in range(PLANES):
            xp, yp, xxp, yyp, xyp = nxt

            # Vs tile: 514 wide, data at cols 1..512, zero pad at 0 and 513;
            # all four streams (x,y,h,g) share one tile so the taps and the
            # paired tail ops run as wide merged ops
            vs_all = vsp.tile([128, 4, NCHUNK, IMG + 2], bf16, tag="vs_all")
            if p < 3:  # zero each pool slot's pad columns once
                nc.vector.memset(vs_all[:, :, :, 0:1], 0.0)
                nc.vector.memset(vs_all[:, :, :, IMG + 1:IMG + 2], 0.0)

            for c in range(NCHUNK):
                V = psum.tile([128, 4, IMG], f32, tag="V")
                # stream order: xy last, so the PE can begin a new plane's
                # chunks before that plane's xy pre-op has finished on V
                if c == 0:
                    band_c = band_b
                    mains = [xp[0:127, 0, :], yp[0:127, 0, :], xxp[0:127, 0, :],
                             yyp[0:127, 0, :], xyp[0:127, 0, :]]
                else:
                    band_c = band_a
                    mains = [xp[:, c, :], yp[:, c, :], xxp[:, c, :],
                             yyp[:, c, :], xyp[:, c, :]]
                if c < NCHUNK - 1:
                    edge_w = e2
                    edges = [xp[0:2, c + 1, :], yp[0:2, c + 1, :],
                             xxp[0:2, c + 1, :], yyp[0:2, c + 1, :],
                             xyp[0:2, c + 1, :]]
                else:
                    edge_w = e1b
                    edges = [xp[64:128, 0, :], yp[64:128, 0, :],
                             xxp[64:128, 0, :], yyp[64:128, 0, :],
                             xyp[64:128, 0, :]]
                # banks: 0=x 1=y 2=xy 3=zz(xx+yy accumulated)
                outs = [V[:, 0, :], V[:, 1, :], V[:, 3, :], V[:, 3, :], V[:, 2, :]]
                for i in range(5):
                    nc.tensor.matmul(outs[i], band_c, mains[i],
                                     start=(i != 3), stop=False)
                for i in range(5):
                    nc.tensor.matmul(outs[i], edge_w, edges[i],
                                     start=False, stop=(i != 2))

                # PSUM->SBUF copies with folded constants (x,y share scale)
                nc.scalar.activation(vs_all[:, 0:2, c, 1:IMG + 1], V[:, 0:2, :],
                                     Act.Copy, scale=RT2)
                nc.scalar.activation(vs_all[:, 2, c, 1:IMG + 1], V[:, 2, :],
                                     Act.Copy, scale=18.0, bias=c2 / 3.0)
                nc.scalar.activation(vs_all[:, 3, c, 1:IMG + 1], V[:, 3, :],
                                     Act.Copy, scale=9.0, bias=c2 / 3.0)


            # horizontal taps for all four streams in one wide op pair
            A = tapA.tile([128, 4, NCHUNK, IMG], bf16, tag="tA")
            nc.vector.tensor_add(A, vs_all[:, :, :, 0:IMG],
                                 vs_all[:, :, :, 2:IMG + 2])
            S_all = smap.tile([128, 4, NCHUNK, IMG], bf16, tag="S")
            nc.vector.tensor_add(S_all, A, vs_all[:, :, :, 1:IMG + 1])
            Sx = S_all[:, 0]
            Sy = S_all[:, 1]
            # Scalar squares issue here; V keeps busy with the pq ops below
            qx = tshort.tile([128, NCHUNK, IMG], bf16, tag="ts")
            nc.scalar.activation(qx, Sx, Act.Square, scale=1.0 / RT2)
            qy = tshort.tile([128, NCHUNK, IMG], bf16, tag="ts")
            nc.scalar.activation(qy, Sy, Act.Square, scale=1.0 / RT2)
            # prefetch + Scalar pre-ops for next plane: after qx/qy so the
            # V-critical squares aren't delayed, before the tail so the PE
            # queue refills in time
            nxt_s = load_pre_s(p + 1) if p + 1 < PLANES else None
            # paired tail over {num,den}: pq={P2,qsum}, nd1=pq+c1,
            # nd2={H,G}-pq, numdd=nd1*nd2 -> {num, dd}
            pq = tmed.tile([128, 2, NCHUNK, IMG], bf16, tag="pq")
            nc.vector.tensor_mul(pq[:, 0], Sx, Sy)
            nc.vector.tensor_add(pq[:, 1], qx, qy)
            nd1 = tmed.tile([128, 2, NCHUNK, IMG], bf16, tag="nd1")
            nc.vector.tensor_scalar_add(nd1, pq, c1)
            nd2 = tmed.tile([128, 2, NCHUNK, IMG], bf16, tag="nd2")
            nc.vector.tensor_sub(nd2, S_all[:, 2:4], pq)
            numdd = tmed.tile([128, 2, NCHUNK, IMG], bf16, tag="numdd")
            nc.vector.tensor_mul(numdd, nd1, nd2)
            # bf16 in/out recip: the DVE pipe upconverts reads to fp32, so
            # the BITWISE_NOT seed still sees an fp32 bit pattern
            from concourse.dve_ops import (RECIP_APPROX_FAST_CONSTS,
                                           RECIPROCAL_APPROX_FAST)
            r = tshort.tile([128, NCHUNK, IMG], bf16, tag="ts")
            _rc = RECIP_APPROX_FAST_CONSTS
            nc.vector._custom_dve(RECIPROCAL_APPROX_FAST, out=r,
                                  in0=numdd[:, 1],
                                  s0=_rc["s0"], s1=_rc["s1"], imm2=_rc["imm2"])
            if nxt_s is not None:
                nxt = pre_v(nxt_s, p + 1)
            sm = tshort.tile([128, NCHUNK, IMG], bf16, tag="ts")
            nc.vector.tensor_mul(sm, numdd[:, 0], r)
            scr = tshort.tile([128, NCHUNK, IMG], bf16, tag="ts")
            nc.scalar.activation(scr, sm, Act.Copy,
                                 accum_out=ssacc[:, p:p + 1])

        red = accs.tile([128, 4], f32, tag="red")
        nc.vector.reduce_sum(red[:, 0:1], xxacc, axis=mybir.AxisListType.X)
        nc.vector.reduce_sum(red[:, 1:2], yyacc, axis=mybir.AxisListType.X)
        nc.vector.reduce_sum(red[:, 2:3], xyacc, axis=mybir.AxisListType.X)
        nc.vector.reduce_sum(red[:, 3:4], ssacc, axis=mybir.AxisListType.X)
        dma.dma_start(out=out_d, in_=red)

    nc.compile()
    return nc


def _band_host():
    b = np.zeros((321, 128), np.float32)
    for i in range(128):            # band_a: k-j in {0,1,2}
        for j in range(128):
            if i - j in (0, 1, 2):
                b[i, j] = 1.0
    for i in range(127):            # band_b: k-j in {-1,0,1}
        for j in range(128):
            if i - j in (-1, 0, 1):
                b[128 + i, j] = 1.0
    b[255, 126] = 1.0               # e2 row 0 (row 128c+127) -> outs 126,127
    b[255, 127] = 1.0
    b[256, 127] = 1.0               # e2 row 1 (row 128c+128) -> out 127
    # e1b (rows 257..320 = lhsT partitions 64..127): only partition 127
    # (= row 511) contributes, to outputs 510 and 511
    b[320, 126] = 1.0
    b[320, 127] = 1.0
    return b


def _get_compiled():
    global _compiled
    if _compiled is None:
        _compiled = _build_nc()
    return _compiled


def _shard_inputs(reconstruction, target):
    import ml_dtypes
    dt = ml_dtypes.bfloat16
    band = _band_host().astype(dt)
    rec = np.asarray(reconstruction).reshape(N_CORES, PLANES, IMG, IMG).astype(dt)
    tgt = np.asarray(target).reshape(N_CORES, PLANES, IMG, IMG).astype(dt)
    return [{"x": np.ascontiguousarray(rec[i]),
             "y": np.ascontiguousarray(tgt[i]),
             "band": band} for i in range(N_CORES)]


def _combine(results):
    sxx = syy = sxy = sss = 0.0
    for i in range(N_CORES):
        red = results[i]["out"].astype(np.float64)
        sxx += red[:, 0].sum()
        syy += red[:, 1].sum()
        sxy += red[:, 2].sum()
        sss += red[:, 3].sum()
    n = float(N_CORES * PLANES * IMG * IMG)
    mse = (sxx + syy - 2.0 * sxy) / n
    ssim_loss = 1.0 - sss / n
    return np.float32(0.8 * mse + 0.2 * ssim_loss)


def run(reconstruction, target, trace=False):
    from concourse.bass_utils import run_bass_kernel_spmd
    nc = _get_compiled()
    in_maps = _shard_inputs(reconstruction, target)
    res = run_bass_kernel_spmd(nc, in_maps, list(range(N_CORES)), trace=trace)
    return _combine(res.results), res


def kernel(reconstruction, target):
    out, _ = run(reconstruction, target, trace=False)
    return out


# revision 29
# speedup vs baseline: 1.0979x; 1.0979x over previous
"""Trainium2 Bass kernel for EnhancedReconstructionLoss (0.8*MSE + 0.2*SSIM-loss).

Sharding: pure data parallel. Batch 32 -> 8 cores x 4 images (12 planes of
512x512 each). Each core computes partial sums (sum x^2, sum y^2, sum x*y,
sum ssim_map); host combines into the scalar loss.

v2 design (vs baseline):
  - Fully-packed input tiles [128, 4, 512]: tile0 = rows 0..126 + row 511 at
    partition 127; tiles 1..3 = rows 127..510. Every pointwise op runs at a
    clean FD=2048 with zero pad waste and exact plane-sum accumulators.
  - zz = xx+yy stream is never materialized: its box filter is computed by
    accumulating the xx and yy matmuls into the same PSUM bank.
  - Vertical 3-tap via banded matmuls (main 127/128-row band + tiny 2-row
    edge accumulation from the next tile; chunk-3 edge reads row 511 from
    tile0 partition 127 via a base-64 quadrant matmul).
  - PSUM->SBUF copies fold all SSIM constants: Sx,Sy scaled by sqrt(2) (so
    P2 = Sx'*Sy' = 2*Sx*Sy), xy scaled by 18 + bias 27*C2, zz scaled by 9 +
    bias 27*C2. After the horizontal taps: H = 18*Sxy+81*C2, G = 9*Szz+81*C2.
  - Tail per plane (FD 2048, all bf16 incl. the reciprocal, emitted via
    _custom_dve to skip the fp32-only wrapper check): qsum=qx+qy,
    den1=qsum+c1, den2=G-qsum, P2, num1=P2+c1, num2=H-P2, num=num1*num2,
    dd=den1*den2, r=recip_approx_fast(dd), sm=num*r, accum.  81^2 scaling
    cancels in the ratio, so ssim values are direct (no host-side x9).
  - Everything elementwise runs on Vector/Scalar only: GpSimd shares the
    SBUF port with the DVE and measurably slows concurrent Vector ops.
  - Emission order is software-pipelined: next plane's DMA + Scalar
    squares are emitted before this plane's tail; the xy pre-op (STT with
    MSE accumulator) is emitted mid-tail so Vector's in-order queue never
    head-of-line blocks on DMA; xy-dependent matmuls go last per chunk.
"""

import sys
import numpy as np

for _p in ("/opt/trn_rl_repo", "/root/.axon_site/_ro/trn_rl_repo"):
    if _p not in sys.path:
        sys.path.insert(0, _p)

N_CORES = 8
IMG = 512
PLANES = 12          # 4 images x 3 channels per core
NCHUNK = 4
C1 = 0.01 ** 2
C2 = 0.03 ** 2
c1 = 81.0 * C1       # folded constants (81 = 9^2 pool divisors, cancels)
c2 = 81.0 * C2
RT2 = float(np.sqrt(2.0))

CFG = {
    "dma_eng": "sync",
}

_compiled = None


def _build_nc():
    from contextlib import ExitStack
    import concourse.bass as bass
    import concourse.tile as tile
    from concourse import bacc, mybir

    f32 = mybir.dt.float32
    bf16 = mybir.dt.bfloat16
    Alu = mybir.AluOpType
    Act = mybir.ActivationFunctionType

    nc = bacc.Bacc("TRN2", target_bir_lowering=False, debug=False,
                   enable_asserts=True, num_devices=N_CORES)
    x_d = nc.dram_tensor("x", [PLANES, IMG, IMG], bf16, kind="ExternalInput").ap()
    y_d = nc.dram_tensor("y", [PLANES, IMG, IMG], bf16, kind="ExternalInput").ap()
    band_d = nc.dram_tensor("band", [321, 128], bf16, kind="ExternalInput").ap()
    out_d = nc.dram_tensor("out", [128, 4], f32, kind="ExternalOutput").ap()

    dma = getattr(nc, CFG["dma_eng"])

    with tile.TileContext(nc) as tc, ExitStack() as ctx:
        consts = ctx.enter_context(tc.tile_pool(name="consts", bufs=1))
        inp = ctx.enter_context(tc.tile_pool(name="inp", bufs=4))
        pre = ctx.enter_context(tc.tile_pool(name="pre", bufs=3))
        psum = ctx.enter_context(tc.tile_pool(name="psum", bufs=2, space="PSUM"))
        vsp = ctx.enter_context(tc.tile_pool(name="vsp", bufs=3))
        smap = ctx.enter_context(tc.tile_pool(name="smap", bufs=2))
        tshort = ctx.enter_context(tc.tile_pool(name="tshort", bufs=4))
        tmed = ctx.enter_context(tc.tile_pool(name="tmed", bufs=1))
        tapA = ctx.enter_context(tc.tile_pool(name="tapA", bufs=2))
        trp = ctx.enter_context(tc.tile_pool(name="trp", bufs=1))
        accs = ctx.enter_context(tc.tile_pool(name="accs", bufs=1))

        # band_a [128,128]: k-j in {0,1,2} (chunks 1..3 main)
        band_a = consts.tile([128, 128], bf16, tag="band_a")
        dma.dma_start(out=band_a, in_=band_d[0:128, :])
        # band_b [127,128]: k-j in {-1,0,1} (chunk 0 main)
        band_b = consts.tile([127, 128], bf16, tag="band_b")
        dma.dma_start(out=band_b, in_=band_d[128:255, :])
        # e2 [2,128]: next-tile rows -> outputs 126,127 (chunks 0..2 edge)
        e2 = consts.tile([2, 128], bf16, tag="e2")
        dma.dma_start(out=e2, in_=band_d[255:257, :])
        # e1b: row 511 (tile0 p127) -> outputs 510,511; lives at partitions
        # 64..127 so lhsT/rhs base partitions match (PE quadrant rule)
        e1b_full = consts.tile([128, 128], bf16, tag="e1b")
        dma.dma_start(out=e1b_full[64:128, :], in_=band_d[257:321, :])
        e1b = e1b_full[64:128, :]

        xxacc = accs.tile([128, PLANES], f32, tag="xxacc")
        yyacc = accs.tile([128, PLANES], f32, tag="yyacc")
        xyacc = accs.tile([128, PLANES], f32, tag="xyacc")
        ssacc = accs.tile([128, PLANES], f32, tag="ssacc")

        def load_plane(dst, src_d, p, eng):
            # tile 0: rows 0..126 at p0..126, row 511 at p127
            eng.dma_start(out=dst[0:127, 0, :], in_=src_d[p, 0:127, :])
            eng.dma_start(out=dst[127:128, 0, :], in_=src_d[p, 511:512, :])
            # tiles 1..2: rows 127..382
            mid = src_d[p, 127:383, :].rearrange("(t r) c -> r t c", r=128)
            eng.dma_start(out=dst[:, 1:3, :], in_=mid)
            # tile 3: rows 383..510
            eng.dma_start(out=dst[:, 3, :], in_=src_d[p, 383:511, :])

        def load_pre_s(p):
            xp = inp.tile([128, NCHUNK, IMG], bf16, tag="xp")
            yp = inp.tile([128, NCHUNK, IMG], bf16, tag="yp")
            load_plane(xp, x_d, p, dma)
            load_plane(yp, y_d, p, dma)
            # pre-pool pointwise; accumulators are exact plane sums (packed)
            xxp = pre.tile([128, NCHUNK, IMG], bf16, tag="xx")
            yyp = pre.tile([128, NCHUNK, IMG], bf16, tag="yy")
            nc.scalar.activation(xxp, xp, Act.Square,
                                 accum_out=xxacc[:, p:p + 1])
            nc.scalar.activation(yyp, yp, Act.Square,
                                 accum_out=yyacc[:, p:p + 1])
            return xp, yp, xxp, yyp

        def pre_v(st, p):
            xp, yp, xxp, yyp = st
            xyp = pre.tile([128, NCHUNK, IMG], bf16, tag="xy")
            nc.vector.scalar_tensor_tensor(
                out=xyp, in0=xp, scalar=1.0, in1=yp,
                op0=Alu.mult, op1=Alu.mult, accum_out=xyacc[:, p:p + 1])
            return xp, yp, xxp, yyp, xyp

        nxt = pre_v(load_pre_s(0), 0)
        for p in range(PLANES):
            xp, yp, xxp, yyp, xyp = nxt

            # Vs tiles: 514 wide, data at cols 1..512, zero pad at 0 and 513;
            # x,y and h,g pairs share one tile so taps run as wide ops
            vs_xy = vsp.tile([128, 2, NCHUNK, IMG + 2], bf16, tag="vs_xy")
            vs_hg = vsp.tile([128, 2, NCHUNK, IMG + 2], bf16, tag="vs_hg")
            if p < 3:  # zero each pool slot's pad columns once
                for t_ in (vs_xy, vs_hg):
                    nc.vector.memset(t_[:, :, :, 0:1], 0.0)
                    nc.vector.memset(t_[:, :, :, IMG + 1:IMG + 2], 0.0)

            for c in range(NCHUNK):
                V = psum.tile([128, 4, IMG], f32, tag="V")
                # stream order: xy last, so the PE can begin a new plane's
                # chunks before that plane's xy pre-op has finished on V
                if c == 0:
                    band_c = band_b
                    mains = [xp[0:127, 0, :], yp[0:127, 0, :], xxp[0:127, 0, :],
                             yyp[0:127, 0, :], xyp[0:127, 0, :]]
                else:
                    band_c = band_a
                    mains = [xp[:, c, :], yp[:, c, :], xxp[:, c, :],
                             yyp[:, c, :], xyp[:, c, :]]
                if c < NCHUNK - 1:
                    edge_w = e2
                    edges = [xp[0:2, c + 1, :], yp[0:2, c + 1, :],
                             xxp[0:2, c + 1, :], yyp[0:2, c + 1, :],
                             xyp[0:2, c + 1, :]]
                else:
                    edge_w = e1b
                    edges = [xp[64:128, 0, :], yp[64:128, 0, :],
                             xxp[64:128, 0, :], yyp[64:128, 0, :],
                             xyp[64:128, 0, :]]
                # banks: 0=x 1=y 2=xy 3=zz(xx+yy accumulated)
                outs = [V[:, 0, :], V[:, 1, :], V[:, 3, :], V[:, 3, :], V[:, 2, :]]
                for i in range(5):
                    nc.tensor.matmul(outs[i], band_c, mains[i],
                                     start=(i != 3), stop=False)
                for i in range(5):
                    nc.tensor.matmul(outs[i], edge_w, edges[i],
                                     start=False, stop=(i != 2))

                # PSUM->SBUF copies with folded constants (x,y share scale)
                nc.scalar.activation(vs_xy[:, :, c, 1:IMG + 1], V[:, 0:2, :],
                                     Act.Copy, scale=RT2)
                nc.scalar.activation(vs_hg[:, 0, c, 1:IMG + 1], V[:, 2, :],
                                     Act.Copy, scale=18.0, bias=c2 / 3.0)
                nc.scalar.activation(vs_hg[:, 1, c, 1:IMG + 1], V[:, 3, :],
                                     Act.Copy, scale=9.0, bias=c2 / 3.0)


            # horizontal taps: S = Vs[j-1] + Vs[j] + Vs[j+1], two streams/op
            def taps(vs, tagp):
                A = tapA.tile([128, 2, NCHUNK, IMG], bf16, tag="tA")
                nc.vector.tensor_add(A, vs[:, :, :, 0:IMG],
                                     vs[:, :, :, 2:IMG + 2])
                S = smap.tile([128, 2, NCHUNK, IMG], bf16, tag="S" + tagp)
                nc.vector.tensor_add(S, A, vs[:, :, :, 1:IMG + 1])
                return S

            S_xy = taps(vs_xy, "xy")
            Sx = S_xy[:, 0]
            Sy = S_xy[:, 1]
            # Scalar squares issue here; V keeps busy with taps/P2 below
            qx = tshort.tile([128, NCHUNK, IMG], bf16, tag="ts")
            nc.scalar.activation(qx, Sx, Act.Square, scale=1.0 / RT2)
            qy = tshort.tile([128, NCHUNK, IMG], bf16, tag="ts")
            nc.scalar.activation(qy, Sy, Act.Square, scale=1.0 / RT2)
            S_hg = taps(vs_hg, "hg")
            H = S_hg[:, 0]
            G = S_hg[:, 1]
            # prefetch + Scalar pre-ops for next plane: after qx/qy so the
            # V-critical squares aren't delayed, before the tail so the PE
            # queue refills in time
            nxt_s = load_pre_s(p + 1) if p + 1 < PLANES else None
            P2 = tmed.tile([128, NCHUNK, IMG], bf16, tag="P2")
            nc.vector.tensor_mul(P2, Sx, Sy)
            num1 = tshort.tile([128, NCHUNK, IMG], bf16, tag="ts")
            nc.vector.tensor_scalar_add(num1, P2, c1)
            num2 = tshort.tile([128, NCHUNK, IMG], bf16, tag="ts")
            nc.vector.tensor_sub(num2, H, P2)
            num = tmed.tile([128, NCHUNK, IMG], bf16, tag="num")
            nc.vector.tensor_mul(num, num1, num2)
            qsum = tmed.tile([128, NCHUNK, IMG], bf16, tag="qsum")
            nc.vector.tensor_add(qsum, qx, qy)
            den1 = tshort.tile([128, NCHUNK, IMG], bf16, tag="ts")
            nc.vector.tensor_scalar_add(den1, qsum, c1)
            den2 = tmed.tile([128, NCHUNK, IMG], bf16, tag="den2")
            nc.vector.tensor_sub(den2, G, qsum)
            dd = tmed.tile([128, NCHUNK, IMG], bf16, tag="dd")
            nc.vector.tensor_mul(dd, den1, den2)
            # bf16 in/out recip: the DVE pipe upconverts reads to fp32, so
            # the BITWISE_NOT seed still sees an fp32 bit pattern
            from concourse.dve_ops import (RECIP_APPROX_FAST_CONSTS,
                                           RECIPROCAL_APPROX_FAST)
            r = trp.tile([128, NCHUNK, IMG], bf16, tag="r")
            _rc = RECIP_APPROX_FAST_CONSTS
            nc.vector._custom_dve(RECIPROCAL_APPROX_FAST, out=r, in0=dd,
                                  s0=_rc["s0"], s1=_rc["s1"], imm2=_rc["imm2"])
            if nxt_s is not None:
                nxt = pre_v(nxt_s, p + 1)
            sm = tshort.tile([128, NCHUNK, IMG], bf16, tag="ts")
            nc.vector.tensor_mul(sm, num, r)
            scr = tshort.tile([128, NCHUNK, IMG], bf16, tag="ts")
            nc.scalar.activation(scr, sm, Act.Copy,
                                 accum_out=ssacc[:, p:p + 1])

        red = accs.tile([128, 4], f32, tag="red")
        nc.vector.reduce_sum(red[:, 0:1], xxacc, axis=mybir.AxisListType.X)
        nc.vector.reduce_sum(red[:, 1:2], yyacc, axis=mybir.AxisListType.X)
        nc.vector.reduce_sum(red[:, 2:3], xyacc, axis=mybir.AxisListType.X)
        nc.vector.reduce_sum(red[:, 3:4], ssacc, axis=mybir.AxisListType.X)
        dma.dma_start(out=out_d, in_=red)

    nc.compile()
    return nc


def _band_host():
    b = np.zeros((321, 128), np.float32)
    for i in range(128):            # band_a: k-j in {0,1,2}
        for j in range(128):
            if i - j in (0, 1, 2):
                b[i, j] = 1.0
    for i in range(127):            # band_b: k-j in {-1,0,1}
        for j in range(128):
            if i - j in (-1, 0, 1):
                b[128 + i, j] = 1.0
    b[255, 126] = 1.0               # e2 row 0 (row 128c+127) -> outs 126,127
    b[255, 127] = 1.0
    b[256, 127] = 1.0               # e2 row 1 (row 128c+128) -> out 127
    # e1b (rows 257..320 = lhsT partitions 64..127): only partition 127
    # (= row 511) contributes, to outputs 510 and 511
    b[320, 126] = 1.0
    b[320, 127] = 1.0
    return b


def _get_compiled():
    global _compiled
    if _compiled is None:
        _compiled = _build_nc()
    return _compiled


def _shard_inputs(reconstruction, target):
    import ml_dtypes
    dt = ml_dtypes.bfloat16
    band = _band_host().astype(dt)
    rec = np.asarray(reconstruction).reshape(N_CORES, PLANES, IMG, IMG).astype(dt)
    tgt = np.asarray(target).reshape(N_CORES, PLANES, IMG, IMG).astype(dt)
    return [{"x": np.ascontiguousarray(rec[i]),
             "y": np.ascontiguousarray(tgt[i]),
             "band": band} for i in range(N_CORES)]


def _combine(results):
    sxx = syy = sxy = sss = 0.0
    for i in range(N_CORES):
        red = results[i]["out"].astype(np.float64)
        sxx += red[:, 0].sum()
        syy += red[:, 1].sum()
        sxy += red[:, 2].sum()
        sss += red[:, 3].sum()
    n = float(N_CORES * PLANES * IMG * IMG)
    mse = (sxx + syy - 2.0 * sxy) / n
    ssim_loss = 1.0 - sss / n
    return np.float32(0.8 * mse + 0.2 * ssim_loss)


def run(reconstruction, target, trace=False):
    from concourse.bass_utils import run_bass_kernel_spmd
    nc = _get_compiled()
    in_maps = _shard_inputs(reconstruction, target)
    res = run_bass_kernel_spmd(nc, in_maps, list(range(N_CORES)), trace=trace)
    return _combine(res.results), res


def kernel(reconstruction, target):
    out, _ = run(reconstruction, target, trace=False)
    return out
